# revision 28
# baseline (speedup 1.0000x reference)
"""Causal self-attention (b=2, n=2048, d=1024, 16 heads) on 8 NeuronCores.

Sharding: core c handles batch b = c // 4 and head group g = c % 4
(heads 4g..4g+3).  qkv weights column-sharded, proj weights row-sharded
(Megatron); each core emits a partial [2048, 1024] proj output (bf16,
scaled by 256) and the host sums the partials, divides by 256 and adds
b_proj.

v2: fp8 DoubleRow matmuls for the projections + QK, bf16 elsewhere.
  - Host ships x and W as fp8 hi+lo pairs (W_qk scaled x32, W_v x256).
    QKV projection runs 3-term DoubleRow (hi*hi + lo*hi + hi*lo), which
    keeps q/k/v near-exact at 4x PE throughput per term.
  - q,k are stored as fp8 [128, slot, n] with head-dims split in two
    32-partition subtiles; QK^T is one DoubleRow matmul per head
    (0.5 cyc/row) -- the only deliberately lossy step (~1.4e-2 rel).
  - exp on ACT emits bf16 et; causal handling is per-128-block: matmul/
    exp start at column 128*t, a single 128-wide bf16 mask strip zeroes
    the sub-diagonal.
  - AV keeps V exact: f32r [V*256|1] stationary x bf16 et moving.
  - normalize: DVE reciprocal + PE ones-broadcast + DVE multiply into
    bf16 onT (no bc copyback; multiply reads PSUM directly).
  - output projection in bf16 (onT x Wp), bf16 y partials DMA'd out.
"""
import sys

sys.path.insert(0, "/opt/trn_rl_repo")

import numpy as np
import ml_dtypes

import concourse.bass as bass  # noqa: F401
import concourse.mybir as mybir
import concourse.tile as tile
from concourse import bacc
from concourse.bass_utils import run_bass_kernel_spmd

F32 = mybir.dt.float32
F32R = mybir.dt.float32r
BF16 = mybir.dt.bfloat16
FP8 = mybir.dt.float8e4
Exp = mybir.ActivationFunctionType.Exp
DRow = mybir.MatmulPerfMode.DoubleRow
FP8NP = ml_dtypes.float8_e4m3

B = 2
N = 2048
D = 1024
NH = 16
HD = 64
NCORES = 8
GROUPS = 4                # head groups (cores per batch)
HPC = NH // GROUPS        # heads per core = 4
PAIRS = HPC // 2          # head pairs per core = 2
QS = 512                  # q_super width
NQS = N // QS             # 4
NB = N // 128             # 16 token blocks
CCH = D // 128            # 8 contraction chunks
SQK = 32.0                # q,k weight scale (fp8 domain)
SV = 256.0                # v weight scale
EXPSCALE = 0.125 / (SQK * SQK)

_CACHE = {}


def _build():
    nc = bacc.Bacc("TRN2", target_bir_lowering=False, debug=False,
                   num_devices=NCORES)
    # host pre-arranges everything partition-major so each DMA is one big
    # contiguous-per-partition transfer (HWDGE charges ~625ns per DMA
    # instruction, so transfer count dominates descriptor time)
    xhi = nc.dram_tensor("xhi", [128, NQS * CCH * QS], FP8,
                         kind="ExternalInput").ap()
    xlo = nc.dram_tensor("xlo", [128, NQS * CCH * QS], FP8,
                         kind="ExternalInput").ap()
    Whi = nc.dram_tensor("Whi", [128, CCH * 768], FP8,
                         kind="ExternalInput").ap()
    Wlo = nc.dram_tensor("Wlo", [128, CCH * 768], FP8,
                         kind="ExternalInput").ap()
    Wp = nc.dram_tensor("Wp", [128, 2 * D], BF16, kind="ExternalInput").ap()
    biasqk = nc.dram_tensor("biasqk", [128, 4], F32, kind="ExternalInput").ap()
    vbias = nc.dram_tensor("vbias", [128, 256], F32, kind="ExternalInput").ap()
    ones64D = nc.dram_tensor("ones64D", [1, 64], F32R, kind="ExternalInput").ap()
    y = nc.dram_tensor("y", [N, D], BF16, kind="ExternalOutput").ap()

    with tile.TileContext(nc) as tc:
        with (
            tc.tile_pool(name="persist", bufs=1) as pp,
            tc.tile_pool(name="xtq_pool", bufs=2) as xtq_pool,
            tc.tile_pool(name="et_pool", bufs=8) as et_pool,
            tc.tile_pool(name="work", bufs=3) as work,
            tc.tile_pool(name="ysb_pool", bufs=6) as ysb_pool,
            tc.tile_pool(name="mm", bufs=2, space="PSUM") as mm,
            tc.tile_pool(name="spool", bufs=2, space="PSUM") as spool,
            tc.tile_pool(name="opool", bufs=2, space="PSUM") as opool,
        ):
            # ---- persistent tiles ----
            Whi_sb = pp.tile([128, CCH, 768], FP8)
            Wlo_sb = pp.tile([128, CCH, 768], FP8)
            Wp_sb = pp.tile([128, 2, D], BF16)
            bqk_sb = pp.tile([128, 4], F32)
            vbias_sb = pp.tile([128, 256], F32)
            ones64 = pp.tile([1, 64], F32R)
            # [head-dim (2 heads x 64), pair, slot, token]; slot 1 is kept
            # all-zero so the DoubleRow second k-subtile contributes 0
            qT8 = pp.tile([128, 2, 2, N], FP8)
            kT8 = pp.tile([128, 2, 2, N], FP8)
            onT = pp.tile([128, 2, N], BF16)  # [pair-head dims, pair, token]
            vaug = pp.tile([128, NB, HPC * 65], BF16)
            vaug_h = vaug.rearrange("p b (h c) -> p b h c", c=65)
            masks = pp.tile([128, 128], BF16)

            Whi_r = Whi.rearrange("p (c f) -> p c f", c=CCH)
            Wlo_r = Wlo.rearrange("p (c f) -> p c f", c=CCH)
            Wp_r = Wp.rearrange("p (c f) -> p c f", c=2)
            xhi_r = xhi.rearrange("p (q c n) -> p q c n", q=NQS, c=CCH)
            xlo_r = xlo.rearrange("p (q c n) -> p q c n", q=NQS, c=CCH)
            y_r = y.rearrange("(t p) f -> t p f", p=128)

            # zero the DoubleRow second subtile slots once (overlaps the
            # initial DMA wait; gpsimd is idle then)
            nc.gpsimd.memset(qT8[:, :, 1, :], 0.0)
            nc.gpsimd.memset(kT8[:, :, 1, :], 0.0)

            # causal mask strip on gpsimd (off the DMA critical path):
            # masks[p, q'] = 1.0 iff q' - p >= 0; it multiplies et columns
            # [128t : 128t+128] of diag block t (local column q' = q - 128t,
            # local key p), which is t-independent
            nc.gpsimd.memset(masks[:], 1.0)
            nc.gpsimd.affine_select(
                out=masks[:],
                in_=masks[:],
                compare_op=mybir.AluOpType.is_ge,
                fill=0.0,
                base=0,
                pattern=[[1, 128]],
                channel_multiplier=-1,
            )

            pending_norm = []

            def emit_norm(pe_bcast=False):
                """normalize deferred (j, hp, o) entries; o entries may be
                PSUM (deferred hp1) or SBUF (drained hp0).  Mid-kernel the
                reciprocal broadcast runs on the idle gpsimd engine; at the
                tail (pe_bcast) the idle PE does it with lower latency."""
                while pending_norm:
                    j, hp, osb = pending_norm.pop(0)
                    if osb[0].space == bass.MemorySpace.PSUM:
                        # the multiply may only read one PSUM operand, so
                        # drain o to SBUF here; this also queues after the
                        # m-tile copybacks on DVE
                        o_ps = osb
                        osb = {}
                        for h in range(2):
                            osb[h] = work.tile([65, QS], F32R, tag="osb",
                                               bufs=4, name=f"osbd{j}{hp}{h}")
                            nc.vector.tensor_copy(osb[h][:], o_ps[h][:])
                    for h in range(2):
                        recip = work.tile([1, QS], F32R, tag="recip",
                                          name=f"r{j}{hp}{h}")
                        with nc.allow_low_precision("f32r recip for bcast"):
                            nc.vector.reciprocal(recip[:], osb[h][64:65, :])
                        if pe_bcast:
                            bc = mm.tile([64, QS], F32, tag="mm",
                                         name=f"bc{j}{hp}{h}")
                            nc.tensor.matmul(bc[:], ones64[:], recip[:],
                                             start=True, stop=True)
                        else:
                            bc = work.tile([64, QS], F32R, tag="bc",
                                           name=f"bc{j}{hp}{h}")
                            nc.gpsimd.partition_broadcast(bc[:], recip[:])
                        nc.vector.tensor_mul(
                            onT[64 * h : 64 * h + 64, hp, QS * j : QS * (j + 1)],
                            osb[h][0:64, :],
                            bc[:],
                        )

            def make_proj_units(jj, tail=False):
                """output projection for quarter jj as one closure per
                (block, half) unit; the two halves share one ysb tile and
                a single whole-block y DMA"""
                ysbs = {}

                def unit(blk, nh):
                    def emit():
                        tb = 4 * jj + blk
                        yps = mm.tile([128, QS], F32, tag="mm",
                                      name=f"y{tb}{nh}")
                        for c in range(2):
                            nc.tensor.matmul(
                                yps[:],
                                onT[:, c, 128 * tb : 128 * (tb + 1)],
                                Wp_sb[:, c, QS * nh : QS * (nh + 1)],
                                start=(c == 0),
                                stop=(c == 1),
                            )
                        if nh == 0:
                            ysbs[blk] = ysb_pool.tile([128, D], BF16,
                                                      tag="ysb",
                                                      name=f"ysb{tb}")
                        ysb = ysbs[blk]
                        # tail: ACT is idle -- alternate copy engines
                        if tail and (blk + nh) % 2 == 1:
                            nc.scalar.copy(ysb[:, QS * nh : QS * (nh + 1)],
                                           yps[:])
                        else:
                            nc.vector.tensor_copy(
                                ysb[:, QS * nh : QS * (nh + 1)], yps[:])
                        if nh == 1:
                            nc.sync.dma_start(y_r[tb], ysb[:])
                    return emit
                return [unit(blk, nh) for blk in range(4) for nh in range(2)]

            def emit_proj(jj, tail=False):
                for u in make_proj_units(jj, tail):
                    u()

            def fetch_xq(q):
                xh = xtq_pool.tile([128, CCH, QS], FP8, tag="xh",
                                   name=f"xh{q}")
                xl = xtq_pool.tile([128, CCH, QS], FP8, tag="xl",
                                   name=f"xl{q}")
                nc.sync.dma_start(xh[:], xhi_r[:, q])
                nc.sync.dma_start(xl[:], xlo_r[:, q])
                return xh, xl

            def emit_mtile_drs(ps, xh, xl, wcol0, wcoln, tokslice=None):
                """the 12 DoubleRow matmuls of a 3-term projection m-tile"""
                terms = ((xh, Whi_sb), (xl, Whi_sb), (xh, Wlo_sb))
                n_mm = 12
                k = 0
                for xt, wt in terms:
                    for cp in range(4):
                        if tokslice is None:
                            rhs = xt[:, 2 * cp : 2 * cp + 2, :]
                            lhsT = wt[:, 2 * cp : 2 * cp + 2, wcol0:wcoln]
                        else:
                            # token-major (V): x stationary, W moving
                            lhsT = xt[:, 2 * cp : 2 * cp + 2, tokslice]
                            rhs = wt[:, 2 * cp : 2 * cp + 2, wcol0:wcoln]
                        nc.tensor.matmul(
                            ps[:], lhsT, rhs,
                            start=(k == 0), stop=(k == n_mm - 1),
                            perf_mode=DRow,
                        )
                        k += 1

            next_xq = None
            for qtr in range(NQS):
                ts, te = QS * qtr, QS * (qtr + 1)
                j = qtr

                # ---- input DMAs, ordered by first consumption ----
                if qtr == 0:
                    xh = xtq_pool.tile([128, CCH, QS], FP8, tag="xh",
                                       name="xh0")
                    xl = xtq_pool.tile([128, CCH, QS], FP8, tag="xl",
                                       name="xl0")
                    nc.sync.dma_start(Whi_sb[:, :, 0:512],
                                      Whi_r[:, :, 0:512])
                    nc.sync.dma_start(xh[:], xhi_r[:, 0])
                    nc.sync.dma_start(Wlo_sb[:, :, 0:512],
                                      Wlo_r[:, :, 0:512])
                    nc.sync.dma_start(xl[:], xlo_r[:, 0])
                    nc.sync.dma_start(bqk_sb[:], biasqk)
                    # ones columns of [V*256|1] on gpsimd
                    nc.gpsimd.memset(vaug_h[:, :, :, 64], 1.0)
                    # v-columns are consumed late (V runs inside hp0's
                    # attention) -- keep them off the critical qk prefix
                    nc.sync.dma_start(Whi_sb[:, :, 512:768],
                                      Whi_r[:, :, 512:768])
                    nc.sync.dma_start(Wlo_sb[:, :, 512:768],
                                      Wlo_r[:, :, 512:768])
                    nc.sync.dma_start(vbias_sb[:], vbias)
                    next_xq = fetch_xq(1)
                    nc.sync.dma_start(ones64[:], ones64D)
                    nc.sync.dma_start(Wp_sb[:], Wp_r[:])
                else:
                    xh, xl = next_xq
                    if qtr + 1 < NQS:
                        next_xq = fetch_xq(qtr + 1)

                # ---- qkv projection m-tiles ----
                # quarter 0: inline, terms-outer per pair-half so the hi*hi
                # matmuls run while the lo DMAs land.  quarters 1..3 were
                # already emitted as fillers inside the previous quarter's
                # i-loop (make_qk_mtile_units), smoothing PE/ACT overlap.
                def make_qk_mtile_units(q, xh_, xl_):
                    tsq, teq = QS * q, QS * (q + 1)

                    def unit(m):
                        def emit():
                            ps = mm.tile([128, QS], F32, tag="mm",
                                         name=f"qk{q}{m}")
                            emit_mtile_drs(ps, xh_, xl_,
                                           128 * m, 128 * (m + 1))
                            dst = qT8 if m < 2 else kT8
                            nc.vector.tensor_scalar_add(
                                dst[:, m % 2, 0, tsq:teq], ps[:],
                                bqk_sb[:, m : m + 1]
                            )
                        return emit
                    return [unit(m) for m in (0, 2, 1, 3)]

                if qtr == 0:
                    for half in ((0, 2), (1, 3)):
                        ps = {
                            m: mm.tile([128, QS], F32, tag="mm",
                                       name=f"qk0{m}")
                            for m in half
                        }
                        terms = ((xh, Whi_sb), (xl, Whi_sb), (xh, Wlo_sb))
                        k = 0
                        for xt, wt in terms:
                            for cp in range(4):
                                for m in half:
                                    nc.tensor.matmul(
                                        ps[m][:],
                                        wt[:, 2 * cp : 2 * cp + 2,
                                           128 * m : 128 * (m + 1)],
                                        xt[:, 2 * cp : 2 * cp + 2, :],
                                        start=(k < len(half)),
                                        stop=(k >= 12 * len(half) - len(half)),
                                        perf_mode=DRow,
                                    )
                                    k += 1
                        for m in half:
                            dst = qT8 if m < 2 else kT8
                            nc.vector.tensor_scalar_add(
                                dst[:, m % 2, 0, ts:te], ps[m][:],
                                bqk_sb[:, m : m + 1]
                            )

                # ---- V token-major (with bias) into [V*256|1] slots ----
                def make_v_units(q=qtr, xh_=xh, xl_=xl):
                    def unit(blk):
                        def emit():
                            tb = 4 * q + blk
                            vps = mm.tile([128, 256], F32, tag="mm",
                                          name=f"v{q}{blk}")
                            emit_mtile_drs(
                                vps, xh_, xl_, 512, 768,
                                tokslice=slice(128 * blk, 128 * (blk + 1)),
                            )
                            nc.vector.tensor_add(
                                vaug_h[:, tb, :, 0:64],
                                vps.rearrange("p (h c) -> p h c", c=64),
                                vbias_sb.rearrange("p (h c) -> p h c", c=64),
                            )
                        return emit
                    return [unit(blk) for blk in range(4)]

                fillers_v = make_v_units()
                fillers_p = []
                if qtr > 0:
                    emit_norm()
                    fillers_p = make_proj_units(qtr - 1)
                if qtr + 1 < NQS:
                    nxh, nxl = next_xq
                    fillers_p = make_qk_mtile_units(qtr + 1, nxh, nxl) \
                        + fillers_p

                # ---- attention for q_super j ----
                n_i = 4 * j + 4
                slots = max(1, 2 * n_i)
                n_fill = len(fillers_v) + len(fillers_p)
                slot = 0
                popped = 0
                for hp in range(PAIRS):
                    if qtr == NQS - 1:
                        emit_norm()
                    o_ps = {
                        h: opool.tile([65, QS], F32, tag="o", name=f"o{j}{hp}{h}")
                        for h in range(2)
                    }
                    ets = {}

                    def emit_qk(i):
                        t = i - 4 * j
                        qs0 = 0 if t < 0 else 128 * t
                        sps = spool.tile([128, 2, QS], F32, tag="s",
                                         name=f"s{j}{hp}{i}")
                        for h in range(2):
                            pb = 64 * h
                            nc.tensor.matmul(
                                sps[:, h, qs0:],
                                kT8[pb : pb + 64, hp, :,
                                    128 * i : 128 * (i + 1)],
                                qT8[pb : pb + 64, hp, :,
                                    QS * j + qs0 : QS * (j + 1)],
                                start=True,
                                stop=True,
                                perf_mode=DRow,
                            )
                        et = et_pool.tile([128, 2, QS], BF16, tag="et",
                                          name=f"et{j}{hp}{i}")
                        nc.scalar.activation(
                            et[:, :, qs0:], sps[:, :, qs0:], Exp,
                            scale=EXPSCALE,
                        )
                        if t >= 0:
                            # only the 128-wide diagonal strip needs masking
                            mhi = min(qs0 + 128, QS)
                            nc.vector.tensor_mul(
                                et[:, :, qs0:mhi],
                                et[:, :, qs0:mhi],
                                masks[:, 0 : mhi - qs0].unsqueeze(1)
                                .broadcast_to([128, 2, mhi - qs0]),
                            )
                        ets[i] = et

                    def emit_av(i):
                        t = i - 4 * j
                        qs0 = 0 if t < 0 else 128 * t
                        et = ets.pop(i)
                        for h in range(2):
                            hh = (2 * hp + h) * 65
                            nc.tensor.matmul(
                                o_ps[h][:, qs0:],
                                vaug[:, i, hh : hh + 65],
                                et[:, h, qs0:],
                                start=(i == 0),
                                stop=(i == n_i - 1),
                            )

                    LOOKAHEAD = 4
                    for i in range(n_i):
                        emit_qk(i)
                        if i >= LOOKAHEAD:
                            emit_av(i - LOOKAHEAD)
                        slot += 1
                        off = 0
                        while (fillers_v or fillers_p) and slot > off and \
                                (slot - off) * n_fill >= \
                                (popped + 1) * max(1, slots - off):
                            popped += 1
                            if fillers_v:
                                fillers_v.pop(0)()
                            else:
                                fillers_p.pop(0)()
                    if hp == 0:
                        while fillers_v:
                            fillers_v.pop(0)()
                    for i in range(max(0, n_i - LOOKAHEAD), n_i):
                        emit_av(i)

                    if hp == PAIRS - 1:
                        # hp1: defer normalize into the flush (straight from
                        # PSUM) so it queues after the m-tile copybacks
                        pending_norm.append((j, hp, o_ps))
                    else:
                        # hp0: drain o to SBUF now to free PSUM for hp1
                        osb = {}
                        for h in range(2):
                            osb[h] = work.tile([65, QS], F32R, tag="osb",
                                               bufs=4, name=f"osb{j}{hp}{h}")
                            nc.vector.tensor_copy(osb[h][:], o_ps[h][:])
                        pending_norm.append((j, hp, osb))

                for u in fillers_p:
                    u()

            emit_norm(pe_bcast=True)
            emit_proj(NQS - 1, tail=True)

    nc.compile()
    return nc


def _host_prep(x, W_qkv, b_qkv, W_proj, b_proj):
    """Build per-core input maps (fp8 hi/lo splits done host-side)."""
    x = np.asarray(x, dtype=np.float32)
    W_qkv = np.asarray(W_qkv, dtype=np.float32)
    b_qkv = np.asarray(b_qkv, dtype=np.float32)
    W_proj = np.asarray(W_proj, dtype=np.float32)

    ones64D = np.ones((1, 64), dtype=np.float32)

    xTs = []
    for b in range(B):
        # [d, n] -> [p, q, c, n'] partition-major: one contiguous 4KB run
        # per (partition, quarter)
        xT = x[b].T.reshape(CCH, 128, NQS, QS).transpose(1, 2, 0, 3)
        xT = np.ascontiguousarray(xT).reshape(128, -1)
        xh = xT.astype(FP8NP)
        xl = (xT - xh.astype(np.float32)).astype(FP8NP)
        xTs.append((xh, xl))

    in_maps = []
    for c in range(NCORES):
        b, g = divmod(c, GROUPS)
        cols = slice(256 * g, 256 * (g + 1))
        Wq = W_qkv[:, 0:1024][:, cols] * SQK
        Wk = W_qkv[:, 1024:2048][:, cols] * SQK
        Wv = W_qkv[:, 2048:3072][:, cols] * SV
        W = np.concatenate([Wq, Wk, Wv], axis=1)
        # [d, f] -> [p, c, f] partition-major
        W = np.ascontiguousarray(
            W.reshape(CCH, 128, 768).transpose(1, 0, 2)
        ).reshape(128, -1)
        Whi = W.astype(FP8NP)
        Wlo = (W - Whi.astype(np.float32)).astype(FP8NP)

        bq = b_qkv[cols.start : cols.stop] * SQK
        bk = b_qkv[1024 + cols.start : 1024 + cols.stop] * SQK
        bv = b_qkv[2048 + cols.start : 2048 + cols.stop] * SV
        biasqk = np.ascontiguousarray(
            np.stack([bq[:128], bq[128:], bk[:128], bk[128:]], axis=1)
        ).astype(np.float32)
        vbias = np.ascontiguousarray(
            np.broadcast_to(bv, (128, 256))
        ).astype(np.float32)
        Wp_slice = np.ascontiguousarray(
            W_proj[cols].reshape(2, 128, D).transpose(1, 0, 2)
        ).reshape(128, -1).astype(ml_dtypes.bfloat16)
        xh, xl = xTs[b]
        in_maps.append(
            {
                "xhi": xh,
                "xlo": xl,
                "Whi": Whi,
                "Wlo": Wlo,
                "Wp": Wp_slice,
                "biasqk": biasqk,
                "vbias": vbias,
                "ones64D": ones64D,
            }
        )
    return in_maps


def _make_runner(nc):
    """Build the PJRT executable once (mirrors bass2jax.run_bass_via_pjrt)
    so repeated kernel() calls skip re-tracing/compile-cache lookups."""
    import jax
    from jax.sharding import Mesh, PartitionSpec
    from jax.experimental.shard_map import shard_map

    from concourse.bass2jax import (
        _bass_exec_p,
        install_neuronx_cc_hook,
        partition_id_tensor,
    )

    install_neuronx_cc_hook()
    partition_name = (
        nc.partition_id_tensor.name if nc.partition_id_tensor else None
    )
    in_names, out_names, out_avals, zero_outs = [], [], [], []
    for alloc in nc.m.functions[0].allocations:
        if not isinstance(alloc, mybir.MemoryLocationSet):
            continue
        name = alloc.memorylocations[0].name
        if alloc.kind == "ExternalInput":
            if name != partition_name:
                in_names.append(name)
        elif alloc.kind == "ExternalOutput":
            out_names.append(name)
            shape = tuple(alloc.tensor_shape)
            dtype = mybir.dt.np(alloc.dtype)
            out_avals.append(jax.core.ShapedArray(shape, dtype))
            zero_outs.append(np.zeros(shape, dtype))
    n_params = len(in_names)
    all_in_names = in_names + out_names
    if partition_name is not None:
        all_in_names = all_in_names + [partition_name]

    def _body(*args):
        operands = list(args)
        if partition_name is not None:
            operands.append(partition_id_tensor())
        return tuple(
            _bass_exec_p.bind(
                *operands,
                out_avals=tuple(out_avals),
                in_names=tuple(all_in_names),
                out_names=tuple(out_names),
                lowering_input_output_aliases=(),
                sim_require_finite=True,
                sim_require_nnan=True,
                nc=nc,
            )
        )

    devices = jax.devices()[:NCORES]
    mesh = Mesh(np.asarray(devices), ("core",))
    in_specs = (PartitionSpec("core"),) * (n_params + len(out_names))
    out_specs = (PartitionSpec("core"),) * len(out_names)
    fn = jax.jit(
        shard_map(_body, mesh=mesh, in_specs=in_specs,
                  out_specs=out_specs, check_rep=False),
        keep_unused=True,
    )
    concat_zeros = [
        np.zeros((NCORES * z.shape[0], *z.shape[1:]), z.dtype)
        for z in zero_outs
    ]

    def run(in_maps):
        concat_in = [
            np.concatenate([np.asarray(m[name]) for m in in_maps], axis=0)
            for name in in_names
        ]
        out_arrs = fn(*concat_in, *concat_zeros)
        return [
            {
                name: np.asarray(out_arrs[i]).reshape(
                    NCORES, *out_avals[i].shape
                )[c]
                for i, name in enumerate(out_names)
            }
            for c in range(NCORES)
        ]

    return run


def kernel(x, W_qkv, b_qkv, W_proj, b_proj):
    if "nc" not in _CACHE:
        _CACHE["nc"] = _build()
        try:
            _CACHE["run"] = _make_runner(_CACHE["nc"])
        except Exception:
            _CACHE["run"] = None
    in_maps = _host_prep(x, W_qkv, b_qkv, W_proj, b_proj)
    results = None
    if _CACHE["run"] is not None:
        try:
            results = _CACHE["run"](in_maps)
        except Exception:
            results = None
    if results is None:
        # fallback: the stock path
        results = run_bass_kernel_spmd(
            _CACHE["nc"], in_maps, core_ids=list(range(NCORES))
        ).results
    out = np.zeros((B, N, D), dtype=np.float32)
    bp = np.asarray(b_proj, dtype=np.float32)
    for b in range(B):
        acc = results[4 * b]["y"].astype(np.float32).copy()
        for g in range(1, GROUPS):
            acc += results[4 * b + g]["y"].astype(np.float32)
        out[b] = acc * (1.0 / SV) + bp
    return out


# revision 31
# speedup vs baseline: 1.0143x; 1.0143x over previous
"""Causal self-attention (b=2, n=2048, d=1024, 16 heads) on 8 NeuronCores.

Sharding: core c handles batch b = c // 4 and head group g = c % 4
(heads 4g..4g+3).  qkv weights column-sharded, proj weights row-sharded
(Megatron); each core emits a partial [2048, 1024] proj output (bf16,
scaled by 256) and the host sums the partials, divides by 256 and adds
b_proj.

v2: fp8 DoubleRow matmuls for the projections + QK, bf16 elsewhere.
  - Host ships x and W as fp8 hi+lo pairs (W_qk scaled x32, W_v x256).
    QKV projection runs 3-term DoubleRow (hi*hi + lo*hi + hi*lo), which
    keeps q/k/v near-exact at 4x PE throughput per term.
  - q,k are stored as fp8 [128, slot, n] with head-dims split in two
    32-partition subtiles; QK^T is one DoubleRow matmul per head
    (0.5 cyc/row) -- the only deliberately lossy step (~1.4e-2 rel).
  - exp on ACT emits bf16 et; causal handling is per-128-block: matmul/
    exp start at column 128*t, a single 128-wide bf16 mask strip zeroes
    the sub-diagonal.
  - AV keeps V exact: f32r [V*256|1] stationary x bf16 et moving.
  - normalize: DVE reciprocal + PE ones-broadcast + DVE multiply into
    bf16 onT (no bc copyback; multiply reads PSUM directly).
  - output projection in bf16 (onT x Wp), bf16 y partials DMA'd out.
"""
import sys

sys.path.insert(0, "/opt/trn_rl_repo")

import numpy as np
import ml_dtypes

import concourse.bass as bass  # noqa: F401
import concourse.mybir as mybir
import concourse.tile as tile
from concourse import bacc
from concourse.bass_utils import run_bass_kernel_spmd

F32 = mybir.dt.float32
F32R = mybir.dt.float32r
BF16 = mybir.dt.bfloat16
FP8 = mybir.dt.float8e4
Exp = mybir.ActivationFunctionType.Exp
DRow = mybir.MatmulPerfMode.DoubleRow
FP8NP = ml_dtypes.float8_e4m3

B = 2
N = 2048
D = 1024
NH = 16
HD = 64
NCORES = 8
GROUPS = 4                # head groups (cores per batch)
HPC = NH // GROUPS        # heads per core = 4
PAIRS = HPC // 2          # head pairs per core = 2
QS = 512                  # q_super width
NQS = N // QS             # 4
NB = N // 128             # 16 token blocks
CCH = D // 128            # 8 contraction chunks
SQK = 32.0                # q,k weight scale (fp8 domain)
SV = 256.0                # v weight scale
EXPSCALE = 0.125 / (SQK * SQK)

_CACHE = {}


def _build():
    nc = bacc.Bacc("TRN2", target_bir_lowering=False, debug=False,
                   num_devices=NCORES)
    # host pre-arranges everything partition-major so each DMA is one big
    # contiguous-per-partition transfer (HWDGE charges ~625ns per DMA
    # instruction, so transfer count dominates descriptor time)
    xhi = nc.dram_tensor("xhi", [128, NQS * CCH * QS], FP8,
                         kind="ExternalInput").ap()
    xlo = nc.dram_tensor("xlo", [128, NQS * CCH * QS], FP8,
                         kind="ExternalInput").ap()
    Whi = nc.dram_tensor("Whi", [128, CCH * 768], FP8,
                         kind="ExternalInput").ap()
    Wlo = nc.dram_tensor("Wlo", [128, CCH * 768], FP8,
                         kind="ExternalInput").ap()
    Wp = nc.dram_tensor("Wp", [128, 2 * D], BF16, kind="ExternalInput").ap()
    biasqk = nc.dram_tensor("biasqk", [128, 4], F32, kind="ExternalInput").ap()
    vbias = nc.dram_tensor("vbias", [128, 256], F32, kind="ExternalInput").ap()
    ones64D = nc.dram_tensor("ones64D", [1, 64], F32R, kind="ExternalInput").ap()
    y = nc.dram_tensor("y", [N, D], BF16, kind="ExternalOutput").ap()

    with tile.TileContext(nc) as tc:
        with (
            tc.tile_pool(name="persist", bufs=1) as pp,
            tc.tile_pool(name="xtq_pool", bufs=2) as xtq_pool,
            tc.tile_pool(name="et_pool", bufs=8) as et_pool,
            tc.tile_pool(name="work", bufs=3) as work,
            tc.tile_pool(name="ysb_pool", bufs=6) as ysb_pool,
            tc.tile_pool(name="mm", bufs=2, space="PSUM") as mm,
            tc.tile_pool(name="spool", bufs=2, space="PSUM") as spool,
            tc.tile_pool(name="opool", bufs=2, space="PSUM") as opool,
        ):
            # ---- persistent tiles ----
            Whi_sb = pp.tile([128, CCH, 768], FP8)
            Wlo_sb = pp.tile([128, CCH, 768], FP8)
            Wp_sb = pp.tile([128, 2, D], BF16)
            bqk_sb = pp.tile([128, 4], F32)
            vbias_sb = pp.tile([128, 256], F32)
            ones64 = pp.tile([1, 64], F32R)
            # [head-dim (2 heads x 64), pair, slot, token]; slot 1 is kept
            # all-zero so the DoubleRow second k-subtile contributes 0
            qT8 = pp.tile([128, 2, 2, N], FP8)
            kT8 = pp.tile([128, 2, 2, N], FP8)
            onT = pp.tile([128, 2, N], BF16)  # [pair-head dims, pair, token]
            vaug = pp.tile([128, NB, HPC * 65], BF16)
            vaug_h = vaug.rearrange("p b (h c) -> p b h c", c=65)
            masks = pp.tile([128, 128], BF16)

            Whi_r = Whi.rearrange("p (c f) -> p c f", c=CCH)
            Wlo_r = Wlo.rearrange("p (c f) -> p c f", c=CCH)
            Wp_r = Wp.rearrange("p (c f) -> p c f", c=2)
            xhi_r = xhi.rearrange("p (q c n) -> p q c n", q=NQS, c=CCH)
            xlo_r = xlo.rearrange("p (q c n) -> p q c n", q=NQS, c=CCH)
            y_r = y.rearrange("(t p) f -> t p f", p=128)

            # zero the DoubleRow second subtile slots once (overlaps the
            # initial DMA wait; gpsimd is idle then)
            nc.gpsimd.memset(qT8[:, :, 1, :], 0.0)
            nc.gpsimd.memset(kT8[:, :, 1, :], 0.0)

            # causal mask strip on gpsimd (off the DMA critical path):
            # masks[p, q'] = 1.0 iff q' - p >= 0; it multiplies et columns
            # [128t : 128t+128] of diag block t (local column q' = q - 128t,
            # local key p), which is t-independent
            nc.gpsimd.memset(masks[:], 1.0)
            nc.gpsimd.affine_select(
                out=masks[:],
                in_=masks[:],
                compare_op=mybir.AluOpType.is_ge,
                fill=0.0,
                base=0,
                pattern=[[1, 128]],
                channel_multiplier=-1,
            )

            pending_norm = []

            def emit_norm(pe_bcast=False):
                """normalize deferred (j, hp, o) entries; o entries may be
                PSUM (deferred hp1) or SBUF (drained hp0).  Mid-kernel the
                reciprocal broadcast runs on the idle gpsimd engine; at the
                tail (pe_bcast) the idle PE does it with lower latency."""
                while pending_norm:
                    j, hp, osb = pending_norm.pop(0)
                    if osb[0].space == bass.MemorySpace.PSUM:
                        # the multiply may only read one PSUM operand, so
                        # drain o to SBUF here; this also queues after the
                        # m-tile copybacks on DVE
                        o_ps = osb
                        osb = {}
                        for h in range(2):
                            osb[h] = work.tile([65, QS], F32R, tag="osb",
                                               bufs=4, name=f"osbd{j}{hp}{h}")
                            nc.vector.tensor_copy(osb[h][:], o_ps[h][:])
                    for h in range(2):
                        recip = work.tile([1, QS], F32R, tag="recip",
                                          name=f"r{j}{hp}{h}")
                        with nc.allow_low_precision("f32r recip for bcast"):
                            nc.vector.reciprocal(recip[:], osb[h][64:65, :])
                        if pe_bcast:
                            bc = mm.tile([64, QS], F32, tag="mm",
                                         name=f"bc{j}{hp}{h}")
                            nc.tensor.matmul(bc[:], ones64[:], recip[:],
                                             start=True, stop=True)
                        else:
                            bc = work.tile([64, QS], F32R, tag="bc",
                                           name=f"bc{j}{hp}{h}")
                            nc.gpsimd.partition_broadcast(bc[:], recip[:])
                        nc.vector.tensor_mul(
                            onT[64 * h : 64 * h + 64, hp, QS * j : QS * (j + 1)],
                            osb[h][0:64, :],
                            bc[:],
                        )

            def make_proj_units(jj, tail=False):
                """output projection for quarter jj as one closure per
                (block, half) unit; the two halves share one ysb tile and
                a single whole-block y DMA"""
                ysbs = {}

                def unit(blk, nh):
                    def emit():
                        tb = 4 * jj + blk
                        yps = mm.tile([128, QS], F32, tag="mm",
                                      name=f"y{tb}{nh}")
                        for c in range(2):
                            nc.tensor.matmul(
                                yps[:],
                                onT[:, c, 128 * tb : 128 * (tb + 1)],
                                Wp_sb[:, c, QS * nh : QS * (nh + 1)],
                                start=(c == 0),
                                stop=(c == 1),
                            )
                        if nh == 0:
                            ysbs[blk] = ysb_pool.tile([128, D], BF16,
                                                      tag="ysb",
                                                      name=f"ysb{tb}")
                        ysb = ysbs[blk]
                        # tail: ACT is idle -- alternate copy engines
                        if tail and (blk + nh) % 2 == 1:
                            nc.scalar.copy(ysb[:, QS * nh : QS * (nh + 1)],
                                           yps[:])
                        else:
                            nc.vector.tensor_copy(
                                ysb[:, QS * nh : QS * (nh + 1)], yps[:])
                        if nh == 1:
                            nc.sync.dma_start(y_r[tb], ysb[:])
                    return emit
                return [unit(blk, nh) for blk in range(4) for nh in range(2)]

            def emit_proj(jj, tail=False):
                for u in make_proj_units(jj, tail):
                    u()

            def fetch_xq(q):
                xh = xtq_pool.tile([128, CCH, QS], FP8, tag="xh",
                                   name=f"xh{q}")
                xl = xtq_pool.tile([128, CCH, QS], FP8, tag="xl",
                                   name=f"xl{q}")
                nc.sync.dma_start(xh[:], xhi_r[:, q])
                nc.sync.dma_start(xl[:], xlo_r[:, q])
                return xh, xl

            def emit_mtile_term(ps, xt, wt, wcol0, wcoln, term_idx,
                                tokslice=None):
                """one term (4 DoubleRow matmuls) of a 3-term projection
                m-tile; term_idx 0 starts the PSUM group, 2 stops it"""
                for cp in range(4):
                    if tokslice is None:
                        rhs = xt[:, 2 * cp : 2 * cp + 2, :]
                        lhsT = wt[:, 2 * cp : 2 * cp + 2, wcol0:wcoln]
                    else:
                        # token-major (V): x stationary, W moving
                        lhsT = xt[:, 2 * cp : 2 * cp + 2, tokslice]
                        rhs = wt[:, 2 * cp : 2 * cp + 2, wcol0:wcoln]
                    nc.tensor.matmul(
                        ps[:], lhsT, rhs,
                        start=(term_idx == 0 and cp == 0),
                        stop=(term_idx == 2 and cp == 3),
                        perf_mode=DRow,
                    )

            def emit_mtile_drs(ps, xh, xl, wcol0, wcoln, tokslice=None):
                terms = ((xh, Whi_sb), (xl, Whi_sb), (xh, Wlo_sb))
                for ti, (xt, wt) in enumerate(terms):
                    emit_mtile_term(ps, xt, wt, wcol0, wcoln, ti, tokslice)

            next_xq = None
            for qtr in range(NQS):
                ts, te = QS * qtr, QS * (qtr + 1)
                j = qtr

                # ---- input DMAs, ordered by first consumption ----
                if qtr == 0:
                    xh = xtq_pool.tile([128, CCH, QS], FP8, tag="xh",
                                       name="xh0")
                    xl = xtq_pool.tile([128, CCH, QS], FP8, tag="xl",
                                       name="xl0")
                    nc.sync.dma_start(Whi_sb[:, :, 0:512],
                                      Whi_r[:, :, 0:512])
                    nc.sync.dma_start(xh[:], xhi_r[:, 0])
                    nc.sync.dma_start(Wlo_sb[:, :, 0:512],
                                      Wlo_r[:, :, 0:512])
                    nc.sync.dma_start(xl[:], xlo_r[:, 0])
                    nc.sync.dma_start(bqk_sb[:], biasqk)
                    # ones columns of [V*256|1] on gpsimd
                    nc.gpsimd.memset(vaug_h[:, :, :, 64], 1.0)
                    # v-columns are consumed late (V runs inside hp0's
                    # attention) -- keep them off the critical qk prefix
                    nc.sync.dma_start(Whi_sb[:, :, 512:768],
                                      Whi_r[:, :, 512:768])
                    nc.sync.dma_start(Wlo_sb[:, :, 512:768],
                                      Wlo_r[:, :, 512:768])
                    nc.sync.dma_start(vbias_sb[:], vbias)
                    next_xq = fetch_xq(1)
                    nc.sync.dma_start(ones64[:], ones64D)
                    nc.sync.dma_start(Wp_sb[:], Wp_r[:])
                else:
                    xh, xl = next_xq
                    if qtr + 1 < NQS:
                        next_xq = fetch_xq(qtr + 1)

                # ---- qkv projection m-tiles ----
                # quarter 0: inline, terms-outer per pair-half so the hi*hi
                # matmuls run while the lo DMAs land.  quarters 1..3 were
                # already emitted as fillers inside the previous quarter's
                # i-loop (make_qk_mtile_units), smoothing PE/ACT overlap.
                def make_qk_mtile_units(q, xh_, xl_):
                    """fine-grained fillers: one sub-unit per term (~430ns
                    of PE) so the QK->exp chain never stalls long"""
                    tsq, teq = QS * q, QS * (q + 1)
                    terms = ((xh_, Whi_sb), (xl_, Whi_sb), (xh_, Wlo_sb))
                    tiles = {}

                    def subunit(m, ti):
                        def emit():
                            if ti == 0:
                                tiles[m] = mm.tile([128, QS], F32, tag="mm",
                                                   name=f"qk{q}{m}")
                            xt, wt = terms[ti]
                            emit_mtile_term(tiles[m], xt, wt,
                                            128 * m, 128 * (m + 1), ti)
                            if ti == 2:
                                dst = qT8 if m < 2 else kT8
                                nc.vector.tensor_scalar_add(
                                    dst[:, m % 2, 0, tsq:teq], tiles[m][:],
                                    bqk_sb[:, m : m + 1]
                                )
                        return emit
                    return [subunit(m, ti) for m in (0, 2, 1, 3)
                            for ti in range(3)]

                if qtr == 0:
                    for half in ((0, 2), (1, 3)):
                        ps = {
                            m: mm.tile([128, QS], F32, tag="mm",
                                       name=f"qk0{m}")
                            for m in half
                        }
                        terms = ((xh, Whi_sb), (xl, Whi_sb), (xh, Wlo_sb))
                        k = 0
                        for xt, wt in terms:
                            for cp in range(4):
                                for m in half:
                                    nc.tensor.matmul(
                                        ps[m][:],
                                        wt[:, 2 * cp : 2 * cp + 2,
                                           128 * m : 128 * (m + 1)],
                                        xt[:, 2 * cp : 2 * cp + 2, :],
                                        start=(k < len(half)),
                                        stop=(k >= 12 * len(half) - len(half)),
                                        perf_mode=DRow,
                                    )
                                    k += 1
                        for m in half:
                            dst = qT8 if m < 2 else kT8
                            nc.vector.tensor_scalar_add(
                                dst[:, m % 2, 0, ts:te], ps[m][:],
                                bqk_sb[:, m : m + 1]
                            )

                # ---- V token-major (with bias) into [V*256|1] slots ----
                def make_v_units(q=qtr, xh_=xh, xl_=xl):
                    terms = ((xh_, Whi_sb), (xl_, Whi_sb), (xh_, Wlo_sb))
                    tiles = {}

                    def subunit(blk, ti):
                        def emit():
                            tb = 4 * q + blk
                            if ti == 0:
                                tiles[blk] = mm.tile([128, 256], F32,
                                                     tag="mm",
                                                     name=f"v{q}{blk}")
                            xt, wt = terms[ti]
                            emit_mtile_term(
                                tiles[blk], xt, wt, 512, 768, ti,
                                tokslice=slice(128 * blk, 128 * (blk + 1)),
                            )
                            if ti == 2:
                                nc.vector.tensor_add(
                                    vaug_h[:, tb, :, 0:64],
                                    tiles[blk].rearrange(
                                        "p (h c) -> p h c", c=64),
                                    vbias_sb.rearrange(
                                        "p (h c) -> p h c", c=64),
                                )
                        return emit
                    return [subunit(blk, ti) for blk in range(4)
                            for ti in range(3)]

                fillers_v = make_v_units()
                fillers_p = []
                if qtr > 0:
                    emit_norm()
                    fillers_p = make_proj_units(qtr - 1)
                if qtr + 1 < NQS:
                    nxh, nxl = next_xq
                    fillers_p = make_qk_mtile_units(qtr + 1, nxh, nxl) \
                        + fillers_p

                # ---- attention for q_super j ----
                n_i = 4 * j + 4
                slots = max(1, 2 * n_i)
                n_fill = len(fillers_v) + len(fillers_p)
                slot = 0
                popped = 0
                for hp in range(PAIRS):
                    if qtr == NQS - 1:
                        emit_norm()
                    o_ps = {
                        h: opool.tile([65, QS], F32, tag="o", name=f"o{j}{hp}{h}")
                        for h in range(2)
                    }
                    ets = {}

                    def emit_qk(i):
                        t = i - 4 * j
                        qs0 = 0 if t < 0 else 128 * t
                        sps = spool.tile([128, 2, QS], F32, tag="s",
                                         name=f"s{j}{hp}{i}")
                        for h in range(2):
                            pb = 64 * h
                            nc.tensor.matmul(
                                sps[:, h, qs0:],
                                kT8[pb : pb + 64, hp, :,
                                    128 * i : 128 * (i + 1)],
                                qT8[pb : pb + 64, hp, :,
                                    QS * j + qs0 : QS * (j + 1)],
                                start=True,
                                stop=True,
                                perf_mode=DRow,
                            )
                        et = et_pool.tile([128, 2, QS], BF16, tag="et",
                                          name=f"et{j}{hp}{i}")
                        nc.scalar.activation(
                            et[:, :, qs0:], sps[:, :, qs0:], Exp,
                            scale=EXPSCALE,
                        )
                        if t >= 0:
                            # only the 128-wide diagonal strip needs masking
                            mhi = min(qs0 + 128, QS)
                            nc.vector.tensor_mul(
                                et[:, :, qs0:mhi],
                                et[:, :, qs0:mhi],
                                masks[:, 0 : mhi - qs0].unsqueeze(1)
                                .broadcast_to([128, 2, mhi - qs0]),
                            )
                        ets[i] = et

                    def emit_av(i):
                        t = i - 4 * j
                        qs0 = 0 if t < 0 else 128 * t
                        et = ets.pop(i)
                        for h in range(2):
                            hh = (2 * hp + h) * 65
                            nc.tensor.matmul(
                                o_ps[h][:, qs0:],
                                vaug[:, i, hh : hh + 65],
                                et[:, h, qs0:],
                                start=(i == 0),
                                stop=(i == n_i - 1),
                            )

                    LOOKAHEAD = 4
                    for i in range(n_i):
                        emit_qk(i)
                        if i >= LOOKAHEAD:
                            emit_av(i - LOOKAHEAD)
                        slot += 1
                        off = 0
                        while (fillers_v or fillers_p) and slot > off and \
                                (slot - off) * n_fill >= \
                                (popped + 1) * max(1, slots - off):
                            popped += 1
                            if fillers_v:
                                fillers_v.pop(0)()
                            else:
                                fillers_p.pop(0)()
                    if hp == 0:
                        while fillers_v:
                            fillers_v.pop(0)()
                    for i in range(max(0, n_i - LOOKAHEAD), n_i):
                        emit_av(i)

                    if hp == PAIRS - 1:
                        # hp1: defer normalize into the flush (straight from
                        # PSUM) so it queues after the m-tile copybacks
                        pending_norm.append((j, hp, o_ps))
                    else:
                        # hp0: drain o to SBUF now to free PSUM for hp1
                        osb = {}
                        for h in range(2):
                            osb[h] = work.tile([65, QS], F32R, tag="osb",
                                               bufs=4, name=f"osb{j}{hp}{h}")
                            nc.vector.tensor_copy(osb[h][:], o_ps[h][:])
                        pending_norm.append((j, hp, osb))

                for u in fillers_p:
                    u()

            emit_norm(pe_bcast=True)
            emit_proj(NQS - 1, tail=True)

    nc.compile()
    return nc


def _host_prep(x, W_qkv, b_qkv, W_proj, b_proj):
    """Build per-core input maps (fp8 hi/lo splits done host-side)."""
    x = np.asarray(x, dtype=np.float32)
    W_qkv = np.asarray(W_qkv, dtype=np.float32)
    b_qkv = np.asarray(b_qkv, dtype=np.float32)
    W_proj = np.asarray(W_proj, dtype=np.float32)

    ones64D = np.ones((1, 64), dtype=np.float32)

    xTs = []
    for b in range(B):
        # [d, n] -> [p, q, c, n'] partition-major: one contiguous 4KB run
        # per (partition, quarter)
        xT = x[b].T.reshape(CCH, 128, NQS, QS).transpose(1, 2, 0, 3)
        xT = np.ascontiguousarray(xT).reshape(128, -1)
        xh = xT.astype(FP8NP)
        xl = (xT - xh.astype(np.float32)).astype(FP8NP)
        xTs.append((xh, xl))

    in_maps = []
    for c in range(NCORES):
        b, g = divmod(c, GROUPS)
        cols = slice(256 * g, 256 * (g + 1))
        Wq = W_qkv[:, 0:1024][:, cols] * SQK
        Wk = W_qkv[:, 1024:2048][:, cols] * SQK
        Wv = W_qkv[:, 2048:3072][:, cols] * SV
        W = np.concatenate([Wq, Wk, Wv], axis=1)
        # [d, f] -> [p, c, f] partition-major
        W = np.ascontiguousarray(
            W.reshape(CCH, 128, 768).transpose(1, 0, 2)
        ).reshape(128, -1)
        Whi = W.astype(FP8NP)
        Wlo = (W - Whi.astype(np.float32)).astype(FP8NP)

        bq = b_qkv[cols.start : cols.stop] * SQK
        bk = b_qkv[1024 + cols.start : 1024 + cols.stop] * SQK
        bv = b_qkv[2048 + cols.start : 2048 + cols.stop] * SV
        biasqk = np.ascontiguousarray(
            np.stack([bq[:128], bq[128:], bk[:128], bk[128:]], axis=1)
        ).astype(np.float32)
        vbias = np.ascontiguousarray(
            np.broadcast_to(bv, (128, 256))
        ).astype(np.float32)
        Wp_slice = np.ascontiguousarray(
            W_proj[cols].reshape(2, 128, D).transpose(1, 0, 2)
        ).reshape(128, -1).astype(ml_dtypes.bfloat16)
        xh, xl = xTs[b]
        in_maps.append(
            {
                "xhi": xh,
                "xlo": xl,
                "Whi": Whi,
                "Wlo": Wlo,
                "Wp": Wp_slice,
                "biasqk": biasqk,
                "vbias": vbias,
                "ones64D": ones64D,
            }
        )
    return in_maps


def _make_runner(nc):
    """Build the PJRT executable once (mirrors bass2jax.run_bass_via_pjrt)
    so repeated kernel() calls skip re-tracing/compile-cache lookups."""
    import jax
    from jax.sharding import Mesh, PartitionSpec
    from jax.experimental.shard_map import shard_map

    from concourse.bass2jax import (
        _bass_exec_p,
        install_neuronx_cc_hook,
        partition_id_tensor,
    )

    install_neuronx_cc_hook()
    partition_name = (
        nc.partition_id_tensor.name if nc.partition_id_tensor else None
    )
    in_names, out_names, out_avals, zero_outs = [], [], [], []
    for alloc in nc.m.functions[0].allocations:
        if not isinstance(alloc, mybir.MemoryLocationSet):
            continue
        name = alloc.memorylocations[0].name
        if alloc.kind == "ExternalInput":
            if name != partition_name:
                in_names.append(name)
        elif alloc.kind == "ExternalOutput":
            out_names.append(name)
            shape = tuple(alloc.tensor_shape)
            dtype = mybir.dt.np(alloc.dtype)
            out_avals.append(jax.core.ShapedArray(shape, dtype))
            zero_outs.append(np.zeros(shape, dtype))
    n_params = len(in_names)
    all_in_names = in_names + out_names
    if partition_name is not None:
        all_in_names = all_in_names + [partition_name]

    def _body(*args):
        operands = list(args)
        if partition_name is not None:
            operands.append(partition_id_tensor())
        return tuple(
            _bass_exec_p.bind(
                *operands,
                out_avals=tuple(out_avals),
                in_names=tuple(all_in_names),
                out_names=tuple(out_names),
                lowering_input_output_aliases=(),
                sim_require_finite=True,
                sim_require_nnan=True,
                nc=nc,
            )
        )

    devices = jax.devices()[:NCORES]
    mesh = Mesh(np.asarray(devices), ("core",))
    in_specs = (PartitionSpec("core"),) * (n_params + len(out_names))
    out_specs = (PartitionSpec("core"),) * len(out_names)
    fn = jax.jit(
        shard_map(_body, mesh=mesh, in_specs=in_specs,
                  out_specs=out_specs, check_rep=False),
        keep_unused=True,
    )
    concat_zeros = [
        np.zeros((NCORES * z.shape[0], *z.shape[1:]), z.dtype)
        for z in zero_outs
    ]

    def run(in_maps):
        concat_in = [
            np.concatenate([np.asarray(m[name]) for m in in_maps], axis=0)
            for name in in_names
        ]
        out_arrs = fn(*concat_in, *concat_zeros)
        return [
            {
                name: np.asarray(out_arrs[i]).reshape(
                    NCORES, *out_avals[i].shape
                )[c]
                for i, name in enumerate(out_names)
            }
            for c in range(NCORES)
        ]

    return run


def kernel(x, W_qkv, b_qkv, W_proj, b_proj):
    if "nc" not in _CACHE:
        _CACHE["nc"] = _build()
        try:
            _CACHE["run"] = _make_runner(_CACHE["nc"])
        except Exception:
            _CACHE["run"] = None
    in_maps = _host_prep(x, W_qkv, b_qkv, W_proj, b_proj)
    results = None
    if _CACHE["run"] is not None:
        try:
            results = _CACHE["run"](in_maps)
        except Exception:
            results = None
    if results is None:
        # fallback: the stock path
        results = run_bass_kernel_spmd(
            _CACHE["nc"], in_maps, core_ids=list(range(NCORES))
        ).results
    out = np.zeros((B, N, D), dtype=np.float32)
    bp = np.asarray(b_proj, dtype=np.float32)
    for b in range(B):
        acc = results[4 * b]["y"].astype(np.float32).copy()
        for g in range(1, GROUPS):
            acc += results[4 * b + g]["y"].astype(np.float32)
        out[b] = acc * (1.0 / SV) + bp
    return out


# revision 34
# speedup vs baseline: 1.0468x; 1.0320x over previous
"""Causal self-attention (b=2, n=2048, d=1024, 16 heads) on 8 NeuronCores.

Sharding: core c handles batch b = c // 4 and head group g = c % 4
(heads 4g..4g+3).  qkv weights column-sharded, proj weights row-sharded
(Megatron); each core emits a partial [2048, 1024] proj output (bf16,
scaled by 256) and the host sums the partials, divides by 256 and adds
b_proj.

v2: fp8 DoubleRow matmuls for the projections + QK, bf16 elsewhere.
  - Host ships x and W as fp8 hi+lo pairs (W_qk scaled x32, W_v x256).
    QKV projection runs 3-term DoubleRow (hi*hi + lo*hi + hi*lo), which
    keeps q/k/v near-exact at 4x PE throughput per term.
  - q,k are stored as fp8 [128, slot, n] with head-dims split in two
    32-partition subtiles; QK^T is one DoubleRow matmul per head
    (0.5 cyc/row) -- the only deliberately lossy step (~1.4e-2 rel).
  - exp on ACT emits bf16 et; causal handling is per-128-block: matmul/
    exp start at column 128*t, a single 128-wide bf16 mask strip zeroes
    the sub-diagonal.
  - AV keeps V exact: f32r [V*256|1] stationary x bf16 et moving.
  - normalize: DVE reciprocal + PE ones-broadcast + DVE multiply into
    bf16 onT (no bc copyback; multiply reads PSUM directly).
  - output projection in bf16 (onT x Wp), bf16 y partials DMA'd out.
"""
import sys

sys.path.insert(0, "/opt/trn_rl_repo")

import numpy as np
import ml_dtypes

import concourse.bass as bass  # noqa: F401
import concourse.mybir as mybir
import concourse.tile as tile
from concourse import bacc
from concourse.bass_utils import run_bass_kernel_spmd

F32 = mybir.dt.float32
F32R = mybir.dt.float32r
BF16 = mybir.dt.bfloat16
FP8 = mybir.dt.float8e4
Exp = mybir.ActivationFunctionType.Exp
DRow = mybir.MatmulPerfMode.DoubleRow
FP8NP = ml_dtypes.float8_e4m3

B = 2
N = 2048
D = 1024
NH = 16
HD = 64
NCORES = 8
GROUPS = 4                # head groups (cores per batch)
HPC = NH // GROUPS        # heads per core = 4
PAIRS = HPC // 2          # head pairs per core = 2
QS = 512                  # q_super width
NQS = N // QS             # 4
NB = N // 128             # 16 token blocks
CCH = D // 128            # 8 contraction chunks
SQK = 32.0                # q,k weight scale (fp8 domain)
SV = 256.0                # v weight scale
EXPSCALE = 0.125 / (SQK * SQK)

_CACHE = {}


def _build():
    nc = bacc.Bacc("TRN2", target_bir_lowering=False, debug=False,
                   num_devices=NCORES)
    # host pre-arranges everything partition-major so each DMA is one big
    # contiguous-per-partition transfer (HWDGE charges ~625ns per DMA
    # instruction, so transfer count dominates descriptor time)
    xhi = nc.dram_tensor("xhi", [128, NQS * CCH * QS], FP8,
                         kind="ExternalInput").ap()
    xlo = nc.dram_tensor("xlo", [128, NQS * CCH * QS], FP8,
                         kind="ExternalInput").ap()
    Whi = nc.dram_tensor("Whi", [128, CCH * 768], FP8,
                         kind="ExternalInput").ap()
    Wlo = nc.dram_tensor("Wlo", [128, CCH * 768], FP8,
                         kind="ExternalInput").ap()
    Wp = nc.dram_tensor("Wp", [128, 2 * D], BF16, kind="ExternalInput").ap()
    biasqk = nc.dram_tensor("biasqk", [128, 4], F32, kind="ExternalInput").ap()
    vbias = nc.dram_tensor("vbias", [128, 256], F32, kind="ExternalInput").ap()
    ones64D = nc.dram_tensor("ones64D", [1, 64], F32R, kind="ExternalInput").ap()
    y = nc.dram_tensor("y", [N, D], BF16, kind="ExternalOutput").ap()

    with tile.TileContext(nc) as tc:
        with (
            tc.tile_pool(name="persist", bufs=1) as pp,
            tc.tile_pool(name="xtq_pool", bufs=2) as xtq_pool,
            tc.tile_pool(name="et_pool", bufs=8) as et_pool,
            tc.tile_pool(name="work", bufs=3) as work,
            tc.tile_pool(name="ysb_pool", bufs=6) as ysb_pool,
            tc.tile_pool(name="mm", bufs=2, space="PSUM") as mm,
            tc.tile_pool(name="spool", bufs=2, space="PSUM") as spool,
            tc.tile_pool(name="opool", bufs=2, space="PSUM") as opool,
        ):
            # ---- persistent tiles ----
            Whi_sb = pp.tile([128, CCH, 768], FP8)
            Wlo_sb = pp.tile([128, CCH, 768], FP8)
            Wp_sb = pp.tile([128, 2, D], BF16)
            bqk_sb = pp.tile([128, 4], F32)
            vbias_sb = pp.tile([128, 256], F32)
            ones64 = pp.tile([1, 64], F32R)
            # [head-dim (2 heads x 64), pair, slot, token]; slot 1 is kept
            # all-zero so the DoubleRow second k-subtile contributes 0
            qT8 = pp.tile([128, 2, 2, N], FP8)
            kT8 = pp.tile([128, 2, 2, N], FP8)
            onT = pp.tile([128, 2, N], BF16)  # [pair-head dims, pair, token]
            vaug = pp.tile([128, NB, HPC * 65], BF16)
            vaug_h = vaug.rearrange("p b (h c) -> p b h c", c=65)
            masks = pp.tile([128, 128], BF16)

            Whi_r = Whi.rearrange("p (c f) -> p c f", c=CCH)
            Wlo_r = Wlo.rearrange("p (c f) -> p c f", c=CCH)
            Wp_r = Wp.rearrange("p (c f) -> p c f", c=2)
            xhi_r = xhi.rearrange("p (q c n) -> p q c n", q=NQS, c=CCH)
            xlo_r = xlo.rearrange("p (q c n) -> p q c n", q=NQS, c=CCH)
            y_r = y.rearrange("(t p) f -> t p f", p=128)

            # zero the DoubleRow second subtile slots once (overlaps the
            # initial DMA wait; gpsimd is idle then)
            nc.gpsimd.memset(qT8[:, :, 1, :], 0.0)
            nc.gpsimd.memset(kT8[:, :, 1, :], 0.0)

            # causal mask strip on gpsimd (off the DMA critical path):
            # masks[p, q'] = 1.0 iff q' - p >= 0; it multiplies et columns
            # [128t : 128t+128] of diag block t (local column q' = q - 128t,
            # local key p), which is t-independent
            nc.gpsimd.memset(masks[:], 1.0)
            nc.gpsimd.affine_select(
                out=masks[:],
                in_=masks[:],
                compare_op=mybir.AluOpType.is_ge,
                fill=0.0,
                base=0,
                pattern=[[1, 128]],
                channel_multiplier=-1,
            )

            pending_norm = []
            proj_queue = []

            def emit_norm(pe_bcast=False):
                """normalize deferred (j, hp, o) entries; o entries may be
                PSUM (deferred hp1) or SBUF (drained hp0).  Mid-kernel the
                reciprocal broadcast runs on the idle gpsimd engine; at the
                tail (pe_bcast) the idle PE does it with lower latency."""
                while pending_norm:
                    j, hp, osb = pending_norm.pop(0)
                    if osb[0].space == bass.MemorySpace.PSUM:
                        # the multiply may only read one PSUM operand, so
                        # drain o to SBUF here; this also queues after the
                        # m-tile copybacks on DVE
                        o_ps = osb
                        osb = {}
                        for h in range(2):
                            osb[h] = work.tile([65, QS], F32R, tag="osb",
                                               bufs=4, name=f"osbd{j}{hp}{h}")
                            nc.vector.tensor_copy(osb[h][:], o_ps[h][:])
                    for h in range(2):
                        recip = work.tile([1, QS], F32R, tag="recip",
                                          name=f"r{j}{hp}{h}")
                        with nc.allow_low_precision("f32r recip for bcast"):
                            nc.vector.reciprocal(recip[:], osb[h][64:65, :])
                        if pe_bcast:
                            bc = mm.tile([64, QS], F32, tag="mm",
                                         name=f"bc{j}{hp}{h}")
                            nc.tensor.matmul(bc[:], ones64[:], recip[:],
                                             start=True, stop=True)
                        else:
                            bc = work.tile([64, QS], F32R, tag="bc",
                                           name=f"bc{j}{hp}{h}")
                            nc.gpsimd.partition_broadcast(bc[:], recip[:])
                        nc.vector.tensor_mul(
                            onT[64 * h : 64 * h + 64, hp, QS * j : QS * (j + 1)],
                            osb[h][0:64, :],
                            bc[:],
                        )

            def make_proj_units(jj, tail=False):
                """output projection for quarter jj as one closure per
                (block, half) unit; the two halves share one ysb tile and
                a single whole-block y DMA"""
                ysbs = {}

                def unit(blk, nh):
                    def emit():
                        tb = 4 * jj + blk
                        yps = mm.tile([128, QS], F32, tag="mm",
                                      name=f"y{tb}{nh}")
                        for c in range(2):
                            nc.tensor.matmul(
                                yps[:],
                                onT[:, c, 128 * tb : 128 * (tb + 1)],
                                Wp_sb[:, c, QS * nh : QS * (nh + 1)],
                                start=(c == 0),
                                stop=(c == 1),
                            )
                        if nh == 0:
                            ysbs[blk] = ysb_pool.tile([128, D], BF16,
                                                      tag="ysb",
                                                      name=f"ysb{tb}")
                        ysb = ysbs[blk]
                        # tail: ACT is idle -- alternate copy engines
                        if tail and (blk + nh) % 2 == 1:
                            nc.scalar.copy(ysb[:, QS * nh : QS * (nh + 1)],
                                           yps[:])
                        else:
                            nc.vector.tensor_copy(
                                ysb[:, QS * nh : QS * (nh + 1)], yps[:])
                        if nh == 1:
                            nc.sync.dma_start(y_r[tb], ysb[:])
                    return emit
                return [unit(blk, nh) for blk in range(4) for nh in range(2)]

            def emit_proj(jj, tail=False):
                for u in make_proj_units(jj, tail):
                    u()

            def fetch_xq(q):
                xh = xtq_pool.tile([128, CCH, QS], FP8, tag="xh",
                                   name=f"xh{q}")
                xl = xtq_pool.tile([128, CCH, QS], FP8, tag="xl",
                                   name=f"xl{q}")
                nc.sync.dma_start(xh[:], xhi_r[:, q])
                nc.sync.dma_start(xl[:], xlo_r[:, q])
                return xh, xl

            def emit_mtile_term(ps, xt, wt, wcol0, wcoln, term_idx,
                                tokslice=None):
                """one term (4 DoubleRow matmuls) of a 3-term projection
                m-tile; term_idx 0 starts the PSUM group, 2 stops it"""
                for cp in range(4):
                    if tokslice is None:
                        rhs = xt[:, 2 * cp : 2 * cp + 2, :]
                        lhsT = wt[:, 2 * cp : 2 * cp + 2, wcol0:wcoln]
                    else:
                        # token-major (V): x stationary, W moving
                        lhsT = xt[:, 2 * cp : 2 * cp + 2, tokslice]
                        rhs = wt[:, 2 * cp : 2 * cp + 2, wcol0:wcoln]
                    nc.tensor.matmul(
                        ps[:], lhsT, rhs,
                        start=(term_idx == 0 and cp == 0),
                        stop=(term_idx == 2 and cp == 3),
                        perf_mode=DRow,
                    )

            def emit_mtile_drs(ps, xh, xl, wcol0, wcoln, tokslice=None):
                terms = ((xh, Whi_sb), (xl, Whi_sb), (xh, Wlo_sb))
                for ti, (xt, wt) in enumerate(terms):
                    emit_mtile_term(ps, xt, wt, wcol0, wcoln, ti, tokslice)

            next_xq = None
            for qtr in range(NQS):
                ts, te = QS * qtr, QS * (qtr + 1)
                j = qtr

                # ---- input DMAs, ordered by first consumption ----
                if qtr == 0:
                    xh = xtq_pool.tile([128, CCH, QS], FP8, tag="xh",
                                       name="xh0")
                    xl = xtq_pool.tile([128, CCH, QS], FP8, tag="xl",
                                       name="xl0")
                    # chunk-halves so the first hi*hi matmuls start ~2us
                    # earlier (the q0 inline emission consumes chunk pairs
                    # in ascending order)
                    nc.sync.dma_start(Whi_sb[:, 0:4, 0:512],
                                      Whi_r[:, 0:4, 0:512])
                    nc.sync.dma_start(xh[:, 0:4, :], xhi_r[:, 0, 0:4])
                    nc.sync.dma_start(Whi_sb[:, 4:8, 0:512],
                                      Whi_r[:, 4:8, 0:512])
                    nc.sync.dma_start(xh[:, 4:8, :], xhi_r[:, 0, 4:8])
                    nc.sync.dma_start(Wlo_sb[:, :, 0:512],
                                      Wlo_r[:, :, 0:512])
                    nc.sync.dma_start(xl[:], xlo_r[:, 0])
                    nc.sync.dma_start(bqk_sb[:], biasqk)
                    # ones columns of [V*256|1] on gpsimd
                    nc.gpsimd.memset(vaug_h[:, :, :, 64], 1.0)
                    # v-columns are consumed late (V runs inside hp0's
                    # attention) -- keep them off the critical qk prefix
                    nc.sync.dma_start(Whi_sb[:, :, 512:768],
                                      Whi_r[:, :, 512:768])
                    nc.sync.dma_start(Wlo_sb[:, :, 512:768],
                                      Wlo_r[:, :, 512:768])
                    nc.sync.dma_start(vbias_sb[:], vbias)
                    next_xq = fetch_xq(1)
                    nc.sync.dma_start(ones64[:], ones64D)
                    nc.sync.dma_start(Wp_sb[:], Wp_r[:])
                else:
                    xh, xl = next_xq
                    if qtr + 1 < NQS:
                        next_xq = fetch_xq(qtr + 1)

                # ---- qkv projection m-tiles ----
                # quarter 0: inline, terms-outer per pair-half so the hi*hi
                # matmuls run while the lo DMAs land.  quarters 1..3 were
                # already emitted as fillers inside the previous quarter's
                # i-loop (make_qk_mtile_units), smoothing PE/ACT overlap.
                def make_qk_mtile_units(q, xh_, xl_):
                    """fine-grained fillers: one sub-unit per term (~430ns
                    of PE) so the QK->exp chain never stalls long"""
                    tsq, teq = QS * q, QS * (q + 1)
                    terms = ((xh_, Whi_sb), (xl_, Whi_sb), (xh_, Wlo_sb))
                    tiles = {}

                    def subunit(m, ti):
                        def emit():
                            if ti == 0:
                                tiles[m] = mm.tile([128, QS], F32, tag="mm",
                                                   name=f"qk{q}{m}")
                            xt, wt = terms[ti]
                            emit_mtile_term(tiles[m], xt, wt,
                                            128 * m, 128 * (m + 1), ti)
                            if ti == 2:
                                dst = qT8 if m < 2 else kT8
                                nc.vector.tensor_scalar_add(
                                    dst[:, m % 2, 0, tsq:teq], tiles[m][:],
                                    bqk_sb[:, m : m + 1]
                                )
                        return emit
                    return [subunit(m, ti) for m in (0, 2, 1, 3)
                            for ti in range(3)]

                if qtr == 0:
                    for half in ((0, 2), (1, 3)):
                        ps = {
                            m: mm.tile([128, QS], F32, tag="mm",
                                       name=f"qk0{m}")
                            for m in half
                        }
                        terms = ((xh, Whi_sb), (xl, Whi_sb), (xh, Wlo_sb))
                        k = 0
                        for xt, wt in terms:
                            for cp in range(4):
                                for m in half:
                                    nc.tensor.matmul(
                                        ps[m][:],
                                        wt[:, 2 * cp : 2 * cp + 2,
                                           128 * m : 128 * (m + 1)],
                                        xt[:, 2 * cp : 2 * cp + 2, :],
                                        start=(k < len(half)),
                                        stop=(k >= 12 * len(half) - len(half)),
                                        perf_mode=DRow,
                                    )
                                    k += 1
                        for m in half:
                            dst = qT8 if m < 2 else kT8
                            nc.vector.tensor_scalar_add(
                                dst[:, m % 2, 0, ts:te], ps[m][:],
                                bqk_sb[:, m : m + 1]
                            )

                # ---- V token-major (with bias) into [V*256|1] slots ----
                def make_v_units(q=qtr, xh_=xh, xl_=xl):
                    terms = ((xh_, Whi_sb), (xl_, Whi_sb), (xh_, Wlo_sb))
                    tiles = {}

                    def subunit(blk, ti):
                        def emit():
                            tb = 4 * q + blk
                            if ti == 0:
                                tiles[blk] = mm.tile([128, 256], F32,
                                                     tag="mm",
                                                     name=f"v{q}{blk}")
                            xt, wt = terms[ti]
                            emit_mtile_term(
                                tiles[blk], xt, wt, 512, 768, ti,
                                tokslice=slice(128 * blk, 128 * (blk + 1)),
                            )
                            if ti == 2:
                                nc.vector.tensor_add(
                                    vaug_h[:, tb, :, 0:64],
                                    tiles[blk].rearrange(
                                        "p (h c) -> p h c", c=64),
                                    vbias_sb.rearrange(
                                        "p (h c) -> p h c", c=64),
                                )
                        return emit
                    return [subunit(blk, ti) for blk in range(4)
                            for ti in range(3)]

                fillers_v = make_v_units()
                fillers_p = []
                if qtr > 0:
                    emit_norm()
                    proj_queue.extend(make_proj_units(qtr - 1))
                # quarters 0/1 are PE-heavy (double m-tile load) while
                # quarter 3's i-loop starves for PE filler work -- defer
                # proj units toward the later, exp-bound quarters
                n_take = {0: 0, 1: 0, 2: 8, 3: len(proj_queue)}[qtr]
                fillers_p = proj_queue[:n_take]
                del proj_queue[:n_take]
                if qtr + 1 < NQS:
                    nxh, nxl = next_xq
                    fillers_p = make_qk_mtile_units(qtr + 1, nxh, nxl) \
                        + fillers_p

                # ---- attention for q_super j ----
                n_i = 4 * j + 4
                slots = max(1, 2 * n_i)
                n_fill = len(fillers_v) + len(fillers_p)
                slot = 0
                popped = 0
                for hp in range(PAIRS):
                    if qtr == NQS - 1:
                        emit_norm()
                    o_ps = {
                        h: opool.tile([65, QS], F32, tag="o", name=f"o{j}{hp}{h}")
                        for h in range(2)
                    }
                    ets = {}

                    def emit_qk(i):
                        t = i - 4 * j
                        qs0 = 0 if t < 0 else 128 * t
                        sps = spool.tile([128, 2, QS], F32, tag="s",
                                         name=f"s{j}{hp}{i}")
                        for h in range(2):
                            pb = 64 * h
                            nc.tensor.matmul(
                                sps[:, h, qs0:],
                                kT8[pb : pb + 64, hp, :,
                                    128 * i : 128 * (i + 1)],
                                qT8[pb : pb + 64, hp, :,
                                    QS * j + qs0 : QS * (j + 1)],
                                start=True,
                                stop=True,
                                perf_mode=DRow,
                            )
                        et = et_pool.tile([128, 2, QS], BF16, tag="et",
                                          name=f"et{j}{hp}{i}")
                        nc.scalar.activation(
                            et[:, :, qs0:], sps[:, :, qs0:], Exp,
                            scale=EXPSCALE,
                        )
                        if t >= 0:
                            # only the 128-wide diagonal strip needs masking
                            mhi = min(qs0 + 128, QS)
                            nc.vector.tensor_mul(
                                et[:, :, qs0:mhi],
                                et[:, :, qs0:mhi],
                                masks[:, 0 : mhi - qs0].unsqueeze(1)
                                .broadcast_to([128, 2, mhi - qs0]),
                            )
                        ets[i] = et

                    def emit_av(i):
                        t = i - 4 * j
                        qs0 = 0 if t < 0 else 128 * t
                        et = ets.pop(i)
                        for h in range(2):
                            hh = (2 * hp + h) * 65
                            nc.tensor.matmul(
                                o_ps[h][:, qs0:],
                                vaug[:, i, hh : hh + 65],
                                et[:, h, qs0:],
                                start=(i == 0),
                                stop=(i == n_i - 1),
                            )

                    LOOKAHEAD = 4
                    for i in range(n_i):
                        emit_qk(i)
                        if i >= LOOKAHEAD:
                            emit_av(i - LOOKAHEAD)
                        slot += 1
                        off = 0
                        while (fillers_v or fillers_p) and slot > off and \
                                (slot - off) * n_fill >= \
                                (popped + 1) * max(1, slots - off):
                            popped += 1
                            if fillers_v:
                                fillers_v.pop(0)()
                            else:
                                fillers_p.pop(0)()
                    if hp == 0:
                        while fillers_v:
                            fillers_v.pop(0)()
                    for i in range(max(0, n_i - LOOKAHEAD), n_i):
                        emit_av(i)

                    if hp == PAIRS - 1:
                        # hp1: defer normalize into the flush (straight from
                        # PSUM) so it queues after the m-tile copybacks
                        pending_norm.append((j, hp, o_ps))
                    else:
                        # hp0: drain o to SBUF now to free PSUM for hp1
                        osb = {}
                        for h in range(2):
                            osb[h] = work.tile([65, QS], F32R, tag="osb",
                                               bufs=4, name=f"osb{j}{hp}{h}")
                            nc.vector.tensor_copy(osb[h][:], o_ps[h][:])
                        pending_norm.append((j, hp, osb))

                for u in fillers_p:
                    u()

            emit_norm(pe_bcast=True)
            emit_proj(NQS - 1, tail=True)

    nc.compile()
    return nc


def _host_prep(x, W_qkv, b_qkv, W_proj, b_proj):
    """Build per-core input maps (fp8 hi/lo splits done host-side)."""
    x = np.asarray(x, dtype=np.float32)
    W_qkv = np.asarray(W_qkv, dtype=np.float32)
    b_qkv = np.asarray(b_qkv, dtype=np.float32)
    W_proj = np.asarray(W_proj, dtype=np.float32)

    ones64D = np.ones((1, 64), dtype=np.float32)

    xTs = []
    for b in range(B):
        # [d, n] -> [p, q, c, n'] partition-major: one contiguous 4KB run
        # per (partition, quarter)
        xT = x[b].T.reshape(CCH, 128, NQS, QS).transpose(1, 2, 0, 3)
        xT = np.ascontiguousarray(xT).reshape(128, -1)
        xh = xT.astype(FP8NP)
        xl = (xT - xh.astype(np.float32)).astype(FP8NP)
        xTs.append((xh, xl))

    in_maps = []
    for c in range(NCORES):
        b, g = divmod(c, GROUPS)
        cols = slice(256 * g, 256 * (g + 1))
        Wq = W_qkv[:, 0:1024][:, cols] * SQK
        Wk = W_qkv[:, 1024:2048][:, cols] * SQK
        Wv = W_qkv[:, 2048:3072][:, cols] * SV
        W = np.concatenate([Wq, Wk, Wv], axis=1)
        # [d, f] -> [p, c, f] partition-major
        W = np.ascontiguousarray(
            W.reshape(CCH, 128, 768).transpose(1, 0, 2)
        ).reshape(128, -1)
        Whi = W.astype(FP8NP)
        Wlo = (W - Whi.astype(np.float32)).astype(FP8NP)

        bq = b_qkv[cols.start : cols.stop] * SQK
        bk = b_qkv[1024 + cols.start : 1024 + cols.stop] * SQK
        bv = b_qkv[2048 + cols.start : 2048 + cols.stop] * SV
        biasqk = np.ascontiguousarray(
            np.stack([bq[:128], bq[128:], bk[:128], bk[128:]], axis=1)
        ).astype(np.float32)
        vbias = np.ascontiguousarray(
            np.broadcast_to(bv, (128, 256))
        ).astype(np.float32)
        Wp_slice = np.ascontiguousarray(
            W_proj[cols].reshape(2, 128, D).transpose(1, 0, 2)
        ).reshape(128, -1).astype(ml_dtypes.bfloat16)
        xh, xl = xTs[b]
        in_maps.append(
            {
                "xhi": xh,
                "xlo": xl,
                "Whi": Whi,
                "Wlo": Wlo,
                "Wp": Wp_slice,
                "biasqk": biasqk,
                "vbias": vbias,
                "ones64D": ones64D,
            }
        )
    return in_maps


def _make_runner(nc):
    """Build the PJRT executable once (mirrors bass2jax.run_bass_via_pjrt)
    so repeated kernel() calls skip re-tracing/compile-cache lookups."""
    import jax
    from jax.sharding import Mesh, PartitionSpec
    from jax.experimental.shard_map import shard_map

    from concourse.bass2jax import (
        _bass_exec_p,
        install_neuronx_cc_hook,
        partition_id_tensor,
    )

    install_neuronx_cc_hook()
    partition_name = (
        nc.partition_id_tensor.name if nc.partition_id_tensor else None
    )
    in_names, out_names, out_avals, zero_outs = [], [], [], []
    for alloc in nc.m.functions[0].allocations:
        if not isinstance(alloc, mybir.MemoryLocationSet):
            continue
        name = alloc.memorylocations[0].name
        if alloc.kind == "ExternalInput":
            if name != partition_name:
                in_names.append(name)
        elif alloc.kind == "ExternalOutput":
            out_names.append(name)
            shape = tuple(alloc.tensor_shape)
            dtype = mybir.dt.np(alloc.dtype)
            out_avals.append(jax.core.ShapedArray(shape, dtype))
            zero_outs.append(np.zeros(shape, dtype))
    n_params = len(in_names)
    all_in_names = in_names + out_names
    if partition_name is not None:
        all_in_names = all_in_names + [partition_name]

    def _body(*args):
        operands = list(args)
        if partition_name is not None:
            operands.append(partition_id_tensor())
        return tuple(
            _bass_exec_p.bind(
                *operands,
                out_avals=tuple(out_avals),
                in_names=tuple(all_in_names),
                out_names=tuple(out_names),
                lowering_input_output_aliases=(),
                sim_require_finite=True,
                sim_require_nnan=True,
                nc=nc,
            )
        )

    devices = jax.devices()[:NCORES]
    mesh = Mesh(np.asarray(devices), ("core",))
    in_specs = (PartitionSpec("core"),) * (n_params + len(out_names))
    out_specs = (PartitionSpec("core"),) * len(out_names)
    fn = jax.jit(
        shard_map(_body, mesh=mesh, in_specs=in_specs,
                  out_specs=out_specs, check_rep=False),
        keep_unused=True,
    )
    concat_zeros = [
        np.zeros((NCORES * z.shape[0], *z.shape[1:]), z.dtype)
        for z in zero_outs
    ]

    def run(in_maps):
        concat_in = [
            np.concatenate([np.asarray(m[name]) for m in in_maps], axis=0)
            for name in in_names
        ]
        out_arrs = fn(*concat_in, *concat_zeros)
        return [
            {
                name: np.asarray(out_arrs[i]).reshape(
                    NCORES, *out_avals[i].shape
                )[c]
                for i, name in enumerate(out_names)
            }
            for c in range(NCORES)
        ]

    return run


def kernel(x, W_qkv, b_qkv, W_proj, b_proj):
    if "nc" not in _CACHE:
        _CACHE["nc"] = _build()
        try:
            _CACHE["run"] = _make_runner(_CACHE["nc"])
        except Exception:
            _CACHE["run"] = None
    in_maps = _host_prep(x, W_qkv, b_qkv, W_proj, b_proj)
    results = None
    if _CACHE["run"] is not None:
        try:
            results = _CACHE["run"](in_maps)
        except Exception:
            results = None
    if results is None:
        # fallback: the stock path
        results = run_bass_kernel_spmd(
            _CACHE["nc"], in_maps, core_ids=list(range(NCORES))
        ).results
    out = np.zeros((B, N, D), dtype=np.float32)
    bp = np.asarray(b_proj, dtype=np.float32)
    for b in range(B):
        acc = results[4 * b]["y"].astype(np.float32).copy()
        for g in range(1, GROUPS):
            acc += results[4 * b + g]["y"].astype(np.float32)
        out[b] = acc * (1.0 / SV) + bp
    return out
